# revision 2
# baseline (speedup 1.0000x reference)
"""3-layer GCN + global mean pool, distributed over 8 Trainium2 NeuronCores.

Optimized v2 (3.0x vs original baseline):

- The per-edge dma_gather is the bottleneck (~8.3ns/row of Q7 descriptor
  emission, serial).  v2 spreads the per-block gathers across 4 SWDGE
  queues (queue q runs on Q7 core pair 2q/2q+1), giving ~3.5x concurrency.
- Per-core true edge counts: each (group, block) gather's index stream is
  real edges followed by -1 padding; the gather ucode strips trailing
  negatives at runtime, so padding costs no descriptor-generation time.
- Edges are packed at (group, block) granularity (GT tiles per group);
  chunks may straddle tile boundaries and are listed in both tiles' stair
  ranges with per-tile dslot columns (-1 for foreign edges).
- Self-loop is added on the TensorEngine (identity matmul against an
  SBUF-resident copy of the core's own u), removing per-tile DMA + DVE add.
- fp16 one-hot builds (2x DVE rate), fp16 table/messages, Shared-addr-space
  collective outputs.
"""

import numpy as np

import concourse.bacc as bacc
import concourse.bass as bass
import concourse.mybir as mybir
import concourse.tile as tile
from concourse.bass_utils import run_bass_kernel_spmd
from concourse.masks import make_identity

P = 128
NCORES = 8
F32 = mybir.dt.float32
F16 = mybir.dt.float16
BF16 = mybir.dt.bfloat16
I16 = mybir.dt.int16

N_GRAPHS = 1000


def _ceil_div(a, b):
    return (a + b - 1) // b


def _preprocess(x, edge_index, batch, n_graphs, group_tiles=5):
    """Host-side edge restructuring.

    Returns per-core input tensors + uniform static structure (meta).
    """
    N = x.shape[0]
    DIN = x.shape[1]
    SR = N // NCORES
    assert SR * NCORES == N
    TPC = _ceil_div(SR, P)        # tiles per core
    S = TPC * P                   # padded shard rows
    NBLK = 4                      # int16 index range: 2 shards per block
    BLKR = 2 * S                  # rows per block
    assert BLKR - 1 <= 32767
    GT = group_tiles
    NG = _ceil_div(TPC, GT)
    GP = _ceil_div(n_graphs, P) * P
    NGT = GP // P

    src = np.asarray(edge_index[0], dtype=np.int64)
    dst = np.asarray(edge_index[1], dtype=np.int64)
    deg = (np.bincount(dst, minlength=N) + 1).astype(np.float32)  # + self
    batch = np.asarray(batch, dtype=np.int64)

    TH = (TPC + 1) // 2           # tiles per half-shard
    SH = TH * P                   # rows per half-shard
    s_core = src // SR
    s_within = src % SR
    s_tile = s_within // P
    s_half = (s_tile >= TH).astype(np.int64)
    rowH = s_core * SH + (s_tile - s_half * TH) * P + (s_within % P)
    dst_core = dst // SR
    dst_slot = dst % SR
    e_tile = dst_slot // P
    e_slot = (dst_slot % P).astype(np.float32)
    e_blk = s_half * 2 + rowH // BLKR     # (half, block-within-half)
    src_row = rowH % BLKR                 # int16 index within block
    e_grp = e_tile // GT

    # ---- pass 1: per-core sorted orders + per-(g,b[,t]) bounds
    NSEG = NG * NBLK
    per_core = []
    seg_len = np.zeros((NCORES, NSEG), dtype=np.int64)
    # chunk range per (core, g, b, t): [lo, hi) in chunks, or empty
    lo_ch = np.full((NCORES, NSEG, TPC), 2**30, dtype=np.int64)
    hi_ch = np.zeros((NCORES, NSEG, TPC), dtype=np.int64)
    for c in range(NCORES):
        m = dst_core == c
        g_c, b_c, t_c = e_grp[m], e_blk[m], e_tile[m]
        order = np.lexsort((t_c, b_c, g_c))
        rows_c = src_row[m][order].astype(np.int16)
        slot_c = e_slot[m][order]
        tile_c = t_c[order]
        kk = ((g_c * NBLK + b_c) * TPC + t_c)[order]
        bounds = np.searchsorted(kk, np.arange(NSEG * TPC + 1))
        per_core.append((rows_c, slot_c, tile_c, bounds))
        for s in range(NSEG):
            s0 = bounds[s * TPC]
            s1 = bounds[(s + 1) * TPC]
            seg_len[c, s] = s1 - s0
            for t in range(s // NBLK * GT, min(s // NBLK * GT + GT, TPC)):
                tlo = bounds[s * TPC + t] - s0
                thi = bounds[s * TPC + t + 1] - s0
                if thi > tlo:
                    lo_ch[c, s, t] = tlo // P
                    hi_ch[c, s, t] = _ceil_div(thi, P)

    nch = _ceil_div(seg_len.max(axis=0), P)          # [NSEG]
    u_lo = lo_ch.min(axis=0)                         # [NSEG, TPC]
    u_hi = hi_ch.max(axis=0)

    # ---- compile-time chunk/dslot structure (uniform across cores)
    # msg chunk layout per group: block-major.  local0[s] = chunk offset of
    # segment s within its group's msg buffer.
    local0 = np.zeros(NSEG, dtype=np.int64)
    ga = np.zeros(NG, dtype=np.int64)
    gb = np.zeros(NG, dtype=np.int64)
    for g in range(NG):
        offa = offb = 0
        for b in range(NBLK):
            if b < 2:
                local0[g * NBLK + b] = offa
                offa += int(nch[g * NBLK + b])
            else:
                local0[g * NBLK + b] = offb
                offb += int(nch[g * NBLK + b])
        ga[g] = offa
        gb[g] = offb
    CHMAXA = int(ga.max())
    CHMAXB = int(gb.max())
    CHMAX = CHMAXA + CHMAXB

    # idx stream offsets (units of 16 idxs)
    idx_off16 = np.zeros(NSEG + 1, dtype=np.int64)
    for s in range(NSEG):
        idx_off16[s + 1] = idx_off16[s] + int(nch[s]) * P // 16
    TOT16 = int(idx_off16[NSEG])

    # per-tile stair structure: tmb (dslot col base), list of
    # (k -> (segment, chunk)) in column order
    tmb = np.zeros(TPC + 1, dtype=np.int64)
    tile_cols = []   # [t] -> list of (seg, chunk_in_seg, local_chunk)
    for t in range(TPC):
        g = t // GT
        cols = []
        for b in range(NBLK):
            s = g * NBLK + b
            if u_lo[s, t] < u_hi[s, t]:
                for k in range(int(u_lo[s, t]), int(u_hi[s, t])):
                    cols.append((s, k, int(local0[s]) + k))
        tile_cols.append(cols)
        tmb[t + 1] = tmb[t] + len(cols)
    DSC = int(tmb[TPC])

    # ---- pass 2: per-core tensors
    in_maps = []
    for c in range(NCORES):
        rows_c, slot_c, tile_c, bounds = per_core[c]
        # pad with row 0 (real data; excluded via dslot=-1).  Negative pads
        # would be stripped by the ucode, desyncing the NX-side ring
        # bookkeeping (reserved from num_idxs_reg) from what the Q7 writes.
        idx_vals = np.zeros(TOT16 * 16, dtype=np.int16)
        for s in range(NSEG):
            s0 = bounds[s * TPC]
            n = int(seg_len[c, s])
            o = int(idx_off16[s]) * 16
            idx_vals[o:o + n] = rows_c[s0:s0 + n]
        dslot_vals = np.full((DSC, P), -1.0, dtype=np.float32)
        for t in range(TPC):
            for ci, (s, k, _lc) in enumerate(tile_cols[t]):
                s0 = bounds[s * TPC]
                n = int(seg_len[c, s])
                p0 = k * P
                p1 = min(p0 + P, n)
                if p1 <= p0:
                    continue
                seg_tiles = tile_c[s0 + p0:s0 + p1]
                seg_slots = slot_c[s0 + p0:s0 + p1]
                col = np.where(seg_tiles == t, seg_slots,
                               np.float32(-1.0)).astype(np.float32)
                dslot_vals[int(tmb[t]) + ci, :p1 - p0] = col
        idx_w = np.tile(idx_vals.reshape(-1, 16).T.copy(), (8, 1))
        dslot_w = dslot_vals.T.copy()                  # [P, DSC]

        nodes = np.arange(c * SR, (c + 1) * SR)
        deg_flat = np.ones(S, dtype=np.float32)
        deg_flat[:SR] = deg[nodes]
        deg_w = deg_flat.reshape(TPC, P).T.copy()
        pool_flat = np.full(S, -1.0, dtype=np.float32)
        pool_flat[:SR] = batch[nodes].astype(np.float32)
        pool_w = pool_flat.reshape(TPC, P).T.copy()

        xT = np.zeros((DIN, S), dtype=np.float32)
        xT[:, :SR] = np.asarray(x[nodes], dtype=np.float32).T

        cnt = np.bincount(batch, minlength=n_graphs).astype(np.float32)
        cnt_flat = np.ones(GP, dtype=np.float32)
        cnt_flat[:n_graphs] = cnt
        cnt_w = cnt_flat[c * P:(c + 1) * P].reshape(1, P).T.copy()

        iota = np.broadcast_to(
            np.arange(GP, dtype=np.float32)[None, :], (P, GP)).copy()

        in_maps.append({
            "xT": xT, "idx": idx_w, "dslot": dslot_w, "deg": deg_w,
            "pslot": pool_w, "cnt": cnt_w, "iota": iota,
        })

    meta = dict(N=N, DIN=DIN, SR=SR, S=S, TPC=TPC, NBLK=NBLK, BLKR=BLKR,
                TH=TH, SH=SH, CHMAXA=CHMAXA, CHMAXB=CHMAXB,
                GT=GT, NG=NG, GP=GP, NGT=NGT, CHMAX=CHMAX, TOT16=TOT16,
                nch=nch, local0=local0, idx_off16=idx_off16, tmb=tmb,
                tile_cols=tile_cols, n_graphs=n_graphs)
    return in_maps, meta


def _build(meta, weights, nqueues=4, stage=4):
    SR, S, TPC = meta["SR"], meta["S"], meta["TPC"]
    NBLK, BLKR = meta["NBLK"], meta["BLKR"]
    GT, NG = meta["GT"], meta["NG"]
    TH, SH = meta["TH"], meta["SH"]
    GP, NGT = meta["GP"], meta["NGT"]
    CHMAX, TOT16 = meta["CHMAX"], meta["TOT16"]
    CHMAXA, CHMAXB = meta["CHMAXA"], meta["CHMAXB"]
    nch, local0, idx_off16 = meta["nch"], meta["local0"], meta["idx_off16"]
    tmb, tile_cols = meta["tmb"], meta["tile_cols"]
    DIN = meta["DIN"]
    H = weights["W1"].shape[1]
    OUT = weights["Wl"].shape[1]
    n_graphs = meta["n_graphs"]
    has_b = [bool(np.any(weights[k])) for k in ("b1", "b2", "b3")]
    has_bl = bool(np.any(weights["bl"]))
    DSC = int(tmb[TPC])

    nc = bacc.Bacc("TRN2", target_bir_lowering=False, debug=False,
                   num_devices=NCORES, num_swdge_queues=nqueues)

    t_xT = nc.dram_tensor("xT", [DIN, S], F32, kind="ExternalInput")
    t_idx = nc.dram_tensor("idx", [P, TOT16], I16, kind="ExternalInput")
    t_dslot = nc.dram_tensor("dslot", [P, DSC], F32, kind="ExternalInput")
    t_deg = nc.dram_tensor("deg", [P, TPC], F32, kind="ExternalInput")
    t_pslot = nc.dram_tensor("pslot", [P, TPC], F32, kind="ExternalInput")
    t_cnt = nc.dram_tensor("cnt", [P, 1], F32, kind="ExternalInput")
    t_iota = nc.dram_tensor("iota", [P, GP], F32, kind="ExternalInput")
    t_W = {}
    for wn, shp in (("W1", [DIN, H]), ("W2", [H, H]), ("W3", [H, H]),
                    ("Wl", [H, OUT])):
        t_W[wn] = nc.dram_tensor(wn, shp, F32, kind="ExternalInput")
    t_b = {}
    for bn in ("b1", "b2", "b3"):
        t_b[bn] = nc.dram_tensor(bn, [P, H], F32, kind="ExternalInput")
    t_bl = nc.dram_tensor("bl", [P, OUT], F32, kind="ExternalInput")
    t_out = nc.dram_tensor("out", [P, OUT], F32, kind="ExternalOutput")

    # internal DRAM: double-buffered (by layer parity) half-shard tables
    sh = [[nc.dram_tensor(f"u_sh{pp}{hh}", [SH, H], BF16, kind="Internal")
           for hh in range(2)] for pp in range(2)]
    tab = [[nc.dram_tensor(f"u_tab{pp}{hh}", [SH * NCORES, H], BF16,
                           kind="Internal")
            for hh in range(2)] for pp in range(2)]
    pool_dram = nc.dram_tensor("pool_dram", [GP, H], F32, kind="Internal")
    pool_rs = nc.dram_tensor("pool_rs", [GP // NCORES, H], F32,
                             kind="Internal")

    AOP = mybir.AluOpType
    ACT = mybir.ActivationFunctionType

    with tile.TileContext(nc, num_cores=NCORES) as tc:
        with tc.tile_pool(name="const", bufs=1) as cp:
            # ---- constants
            iota_sb = cp.tile([P, GP], F32)
            nc.sync.dma_start(iota_sb[:], t_iota[:])
            idx_sb = cp.tile([P, TOT16], I16)
            nc.sync.dma_start(idx_sb[:], t_idx[:])
            dslot32_sb = cp.tile([P, DSC], F32)
            nc.sync.dma_start(dslot32_sb[:], t_dslot[:])
            dslot_sb = cp.tile([P, DSC], BF16)
            nc.vector.tensor_copy(dslot_sb[:], dslot32_sb[:])
            iota_bf = cp.tile([P, P], BF16)
            nc.vector.tensor_copy(iota_bf[:], iota_sb[:, 0:P])
            deg_sb = cp.tile([P, TPC], F32)
            nc.sync.dma_start(deg_sb[:], t_deg[:])
            pslot_sb = cp.tile([P, TPC], F32)
            nc.sync.dma_start(pslot_sb[:], t_pslot[:])
            cnt_sb = cp.tile([P, 1], F32)
            nc.sync.dma_start(cnt_sb[:], t_cnt[:])
            W_sb = {}
            for wn, t_w in t_W.items():
                W_sb[wn] = cp.tile(list(t_w.shape), F32, name=f"W_{wn}_sb")
                nc.sync.dma_start(W_sb[wn][:], t_w[:])
            # fp16 copies of W2/W3 for the in-loop transform
            W16 = {}
            for wn in ("W2", "W3"):
                W16[wn] = cp.tile([H, H], BF16, name=f"W16_{wn}_sb")
                nc.vector.tensor_copy(W16[wn][:], W_sb[wn][:])
            b_sb = {}
            for i, bn in enumerate(("b1", "b2", "b3")):
                if has_b[i]:
                    b_sb[bn] = cp.tile([P, H], F32, name=f"b_{bn}_sb")
                    nc.sync.dma_start(b_sb[bn][:], t_b[bn][:])
            if has_bl:
                bl_sb = cp.tile([P, OUT], F32)
                nc.sync.dma_start(bl_sb[:], t_bl[:])
            ident32 = cp.tile([P, P], F32)
            make_identity(nc, ident32[:])
            ident16 = cp.tile([P, P], BF16)
            nc.vector.tensor_copy(ident16[:], ident32[:])

            dinv_sb = cp.tile([P, TPC], F32)
            nc.scalar.sqrt(dinv_sb[:], deg_sb[:])
            nc.vector.reciprocal(dinv_sb[:], dinv_sb[:])
            cntinv_sb = cp.tile([P, 1], F32)
            nc.vector.tensor_scalar_max(cntinv_sb[:], cnt_sb[:], 1.0)
            nc.vector.reciprocal(cntinv_sb[:], cntinv_sb[:])

            # own-shard u, resident in SBUF (also mirrored to u_shard DRAM)
            u_own = cp.tile([P, TPC, H], BF16)

            # ---- layer-1 u: u1 = dinv * (x @ W1)
            with tc.tile_pool(name="xTp", bufs=1) as xp, \
                 tc.tile_pool(name="u1ps", bufs=4, space="PSUM") as u1ps:
                xT_sb = xp.tile([DIN, S], F32)
                nc.sync.dma_start(xT_sb[:], t_xT[:])
                for t in range(TPC):
                    ps = u1ps.tile([P, H], F32, tag="ps")
                    nc.tensor.matmul(ps[:], lhsT=xT_sb[:, t * P:(t + 1) * P],
                                     rhs=W_sb["W1"][:], start=True, stop=True)
                    nc.scalar.activation(u_own[:, t, :], ps[:], ACT.Copy,
                                         scale=dinv_sb[:, t:t + 1])
                    hh = int(t >= TH)
                    tt = t - hh * TH
                    nc.sync.dma_start(sh[0][hh][tt * P:(tt + 1) * P, :],
                                      u_own[:, t, :])
                    if stage >= 2 and t in (TH - 1, TPC - 1):
                        nc.gpsimd.collective_compute(
                            "AllGather", AOP.bypass,
                            replica_groups=[list(range(NCORES))],
                            ins=[sh[0][hh][:]], outs=[tab[0][hh][:]],
                        )

            # ---- main layer loop
            with tc.tile_pool(name="msga", bufs=4) as mpa, \
                 tc.tile_pool(name="msgb", bufs=3) as mpb, \
                 tc.tile_pool(name="stair", bufs=3) as sp, \
                 tc.tile_pool(name="work", bufs=3) as wp, \
                 tc.tile_pool(name="hps", bufs=2, space="PSUM") as hps, \
                 tc.tile_pool(name="tps", bufs=2, space="PSUM") as tps, \
                 tc.tile_pool(name="ups", bufs=2, space="PSUM") as ups, \
                 tc.tile_pool(name="pps", bufs=1, space="PSUM") as pps:
                pool_ps = pps.tile([P, GP], F32)
                # zero-init all msg buffers once (pad chunks are read by
                # the stair matmuls; stale SBUF could be NaN)
                for _ in range(4):
                    mz = mpa.tile([P, CHMAXA, H], BF16, tag="msga")
                    nc.vector.memset(mz[:], 0.0)
                for _ in range(3):
                    mz = mpb.tile([P, CHMAXB, H], BF16, tag="msgb")
                    nc.vector.memset(mz[:], 0.0)

                n_layers = 3 if stage >= 2 else 0
                for li in range(n_layers):
                    rpar = li % 2         # tables read this layer
                    wpar = (li + 1) % 2   # shards/tables produced this layer
                    W_next = ("W2", "W3", None)[li]
                    LA = 3
                    msgsA, msgsB = {}, {}

                    def issue(g, blist, mtile):
                        for b in blist:
                            s = g * NBLK + b
                            n_idx = int(nch[s]) * P
                            if n_idx == 0:
                                continue
                            nc.gpsimd.dma_gather(
                                out_ap=mtile[:, int(local0[s]):
                                             int(local0[s]) + int(nch[s]), :],
                                in_ap=tab[rpar][b // 2][
                                    (b % 2) * BLKR:(b % 2 + 1) * BLKR, :],
                                idxs_ap=idx_sb[:, int(idx_off16[s]):
                                               int(idx_off16[s + 1])],
                                num_idxs=n_idx,
                                num_idxs_reg=n_idx,
                                elem_size=H,
                                single_packet=False,
                                queue_num=b % nqueues,
                            )

                    for gi in range(NG + LA):
                        if gi < NG:
                            msgsA[gi] = mpa.tile([P, CHMAXA, H], BF16,
                                                 tag="msga",
                                                 name=f"msga_{li}_{gi}")
                            issue(gi, (0, 1), msgsA[gi])
                        gB = gi - LA + 1
                        if 0 <= gB < NG:
                            msgsB[gB] = mpb.tile([P, CHMAXB, H], BF16,
                                                 tag="msgb",
                                                 name=f"msgb_{li}_{gB}")
                            issue(gB, (2, 3), msgsB[gB])
                        g = gi - LA
                        if g < 0:
                            continue
                        msgA = msgsA.pop(g)
                        msgB = msgsB.pop(g)
                        for t in range(g * GT, min((g + 1) * GT, TPC)):
                            if stage < 3:
                                continue
                            cols = tile_cols[t]
                            ntc = len(cols)
                            ps_h = hps.tile([P, H], F32, tag="h")
                            if ntc:
                                stair = sp.tile([P, ntc, P], BF16, tag="st")
                                nc.vector.tensor_tensor(
                                    out=stair[:],
                                    in0=iota_bf[:].unsqueeze(1)
                                        .to_broadcast([P, ntc, P]),
                                    in1=dslot_sb[:, int(tmb[t]):
                                                 int(tmb[t]) + ntc]
                                        .unsqueeze(2)
                                        .to_broadcast([P, ntc, P]),
                                    op=AOP.is_equal,
                                )
                                for k, (_s, _k, lc) in enumerate(cols):
                                    mt = msgA if _s % NBLK < 2 else msgB
                                    nc.tensor.matmul(
                                        ps_h[:], lhsT=stair[:, k, :],
                                        rhs=mt[:, lc, :],
                                        start=(k == 0), stop=False)
                            # self-loop via identity matmul
                            nc.tensor.matmul(
                                ps_h[:], lhsT=ident16[:],
                                rhs=u_own[:, t, :],
                                start=(ntc == 0), stop=True)
                            h_sb = wp.tile([P, H], BF16, tag="h")
                            if has_b[li]:
                                tmp = wp.tile([P, H], F32, tag="tmp")
                                nc.vector.tensor_scalar_mul(
                                    tmp[:], ps_h[:], dinv_sb[:, t:t + 1])
                                nc.vector.tensor_tensor(
                                    out=tmp[:], in0=tmp[:],
                                    in1=b_sb[("b1", "b2", "b3")[li]][:],
                                    op=AOP.add)
                                nc.scalar.activation(h_sb[:], tmp[:],
                                                     ACT.Relu)
                            else:
                                nc.scalar.activation(
                                    h_sb[:], ps_h[:], ACT.Relu,
                                    scale=dinv_sb[:, t:t + 1])
                            if W_next is not None:
                                ps_t = tps.tile([P, P], BF16, tag="t")
                                nc.tensor.transpose(ps_t[:], h_sb[:],
                                                    ident16[:])
                                hT_sb = wp.tile([P, P], BF16, tag="ht")
                                nc.vector.tensor_copy(hT_sb[:], ps_t[:])
                                ps_u = ups.tile([P, H], F32, tag="u")
                                nc.tensor.matmul(ps_u[:], lhsT=hT_sb[:],
                                                 rhs=W16[W_next][:],
                                                 start=True, stop=True)
                                nc.scalar.activation(
                                    u_own[:, t, :], ps_u[:], ACT.Copy,
                                    scale=dinv_sb[:, t:t + 1])
                                hh2 = int(t >= TH)
                                tt2 = t - hh2 * TH
                                nc.sync.dma_start(
                                    sh[wpar][hh2][tt2 * P:(tt2 + 1) * P, :],
                                    u_own[:, t, :])
                                if t in (TH - 1, TPC - 1):
                                    nc.gpsimd.collective_compute(
                                        "AllGather", AOP.bypass,
                                        replica_groups=[
                                            list(range(NCORES))],
                                        ins=[sh[wpar][hh2][:]],
                                        outs=[tab[wpar][hh2][:]],
                                    )
                            else:
                                stp = sp.tile([P, GP], BF16, tag="stp")
                                nc.vector.tensor_tensor(
                                    out=stp[:], in0=iota_sb[:],
                                    in1=pslot_sb[:, t:t + 1]
                                        .to_broadcast([P, GP]),
                                    op=AOP.is_equal)
                                for hh in range(NGT):
                                    nc.tensor.matmul(
                                        pool_ps[:, hh * P:(hh + 1) * P],
                                        lhsT=stp[:, hh * P:(hh + 1) * P],
                                        rhs=h_sb[:],
                                        start=(t == 0 and hh % 4 == 0),
                                        stop=(t == TPC - 1),
                                        skip_group_check=True)

                # ---- pool wrap-up
                if stage < 4:
                    z_sb = wp.tile([P, GP], F32, tag="pa")
                    nc.vector.memset(z_sb[:], 0.0)
                    nc.sync.dma_start(t_out[:], z_sb[:, :OUT])
                else:
                    poolacc = wp.tile([P, GP], F32, tag="pa")
                    nc.vector.tensor_copy(poolacc[:], pool_ps[:])
                    nc.sync.dma_start(
                        pool_dram[:].rearrange("(h p) f -> p h f", p=P),
                        poolacc[:].rearrange("p (h f) -> p h f", h=NGT))
                    # each core keeps its own 128-graph block, fully summed
                    nc.gpsimd.collective_compute(
                        "ReduceScatter", AOP.add,
                        replica_groups=[list(range(NCORES))],
                        ins=[pool_dram[:]], outs=[pool_rs[:]],
                    )
                    pt = wp.tile([P, H], F32, tag="pt")
                    nc.sync.dma_start(pt[:], pool_rs[:])
                    nc.vector.tensor_scalar_mul(pt[:], pt[:],
                                                cntinv_sb[:, 0:1])
                    ps_t = tps.tile([P, P], F32, tag="t")
                    nc.tensor.transpose(ps_t[:], pt[:], ident32[:])
                    ptT = wp.tile([P, P], F32, tag="ptT")
                    nc.vector.tensor_copy(ptT[:], ps_t[:])
                    ps_o = ups.tile([P, OUT], F32, tag="u")
                    nc.tensor.matmul(ps_o[:], lhsT=ptT[:],
                                     rhs=W_sb["Wl"][:],
                                     start=True, stop=True)
                    o_sb = wp.tile([P, OUT], F32, tag="o")
                    if has_bl:
                        nc.vector.tensor_tensor(out=o_sb[:], in0=ps_o[:],
                                                in1=bl_sb[:], op=AOP.add)
                    else:
                        nc.vector.tensor_copy(o_sb[:], ps_o[:])
                    nc.sync.dma_start(t_out[:], o_sb[:])

    nc.finalize()
    return nc


def kernel(x, edge_index, batch, W1, b1, W2, b2, W3, b3, Wl, bl,
           group_tiles=5, trace=False, n_graphs=N_GRAPHS, nqueues=4, stage=4):
    weights = dict(W1=np.asarray(W1, np.float32), b1=np.asarray(b1, np.float32),
                   W2=np.asarray(W2, np.float32), b2=np.asarray(b2, np.float32),
                   W3=np.asarray(W3, np.float32), b3=np.asarray(b3, np.float32),
                   Wl=np.asarray(Wl, np.float32), bl=np.asarray(bl, np.float32))
    in_maps, meta = _preprocess(np.asarray(x, np.float32),
                                np.asarray(edge_index), np.asarray(batch),
                                n_graphs, group_tiles=group_tiles)
    nc = _build(meta, weights, nqueues=nqueues, stage=stage)
    H = weights["W1"].shape[1]
    OUT = weights["Wl"].shape[1]
    for m in in_maps:
        for wn in ("W1", "W2", "W3", "Wl"):
            m[wn] = weights[wn]
        for bn in ("b1", "b2", "b3"):
            m[bn] = np.broadcast_to(weights[bn][None, :], (P, H)).copy()
        m["bl"] = np.broadcast_to(weights["bl"][None, :], (P, OUT)).copy()
    res = run_bass_kernel_spmd(nc, in_maps, core_ids=list(range(NCORES)),
                               trace=trace)
    kernel.last_result = res
    GP = _ceil_div(n_graphs, P) * P
    out = np.zeros((n_graphs, OUT), dtype=np.float32)
    for c in range(NCORES):
        rows = min(P, n_graphs - c * P)
        if rows > 0:
            out[c * P:c * P + rows] = np.asarray(
                res.results[c]["out"][:rows], dtype=np.float32)
    return out
